# revision 14
# baseline (speedup 1.0000x reference)
"""Multi-head attention (B=8, N=1024, C=1024, H=16) on 8 TRN2 NeuronCores.

Strategy: pure data parallelism -- each core computes one batch element with
replicated weights (no collectives). All matmul operands are bf16 (PSUM
accumulation stays fp32), which halves HBM/SBUF traffic and keeps every
tensor resident so the phases can be globally interleaved.

Per-core layout (everything transposed so matmuls contract on partitions):
  xT  [C, N]   = x[b].T          wqT/wkT/wvT [C, C] = in_proj slices .T
  woT [C, C]   = out_proj_weight.T

Emission order = scheduler priority. The exp stream on the Scalar engine
(143us total) is the second-longest resource after the tensor engine
(~190us), so attention pairs are emitted early and projection / out-proj
matmuls are placed after each pair as fillers that soak up the tensor
engine whenever attention is waiting on exp:

  B(k0) B(q0) | [A(V) woven into pair0's units] | pair0 | B(k1) B(q1) |
  pair1 | B(k2) B(q2) | ... | pair7.ic0 | D(nch0) | pair7.ic1 | D(nch1)

Attention per pair hp (heads 2hp, 2hp+1), ic-major (query chunks of 512):
  QK: two row-group-concurrent matmuls (contraction 64) -> S [128k, 1024]
  exp: one ACT op per unit, bf16 out, scale folded in
  PV: V_hat [128k, 65] (64 dims + ones col -> softmax sums in row 64)
  norm: copy out of PSUM, reciprocal of row 64, gpsimd partition
        broadcast, multiply into A_sb (odd heads DMA-shifted to rows 64+)
  D:  out[cb, nch] = sum_hb woT_hb.T @ A_hb + bias
"""
import numpy as np

B, N, C = 8, 1024, 1024
H = 16
HD = C // H               # 64
SCALE = HD ** (-0.5)
NCORES = 8

_COMPILED = {}


def _build():
    import concourse.bass as bass
    import concourse.tile as tile
    from concourse import bacc, mybir

    F32 = mybir.dt.float32
    BF16 = mybir.dt.bfloat16
    EXP = mybir.ActivationFunctionType.Exp

    nc = bacc.Bacc("TRN2", target_bir_lowering=False, debug=False)

    xT = nc.dram_tensor("xT", [C, N], BF16, kind="ExternalInput").ap()
    wqT = nc.dram_tensor("wqT", [C, C], BF16, kind="ExternalInput").ap()
    wkT = nc.dram_tensor("wkT", [C, C], BF16, kind="ExternalInput").ap()
    wvT = nc.dram_tensor("wvT", [C, C], BF16, kind="ExternalInput").ap()
    woT = nc.dram_tensor("woT", [C, C], BF16, kind="ExternalInput").ap()
    bqk = nc.dram_tensor("bqk", [128, 16], F32, kind="ExternalInput").ap()
    bv = nc.dram_tensor("bv", [1, C], F32, kind="ExternalInput").ap()
    bo = nc.dram_tensor("bo", [128, 8], F32, kind="ExternalInput").ap()
    ones_col = nc.dram_tensor("ones_col", [128, 16], BF16, kind="ExternalInput").ap()
    outT = nc.dram_tensor("outT", [C, N], F32, kind="ExternalOutput").ap()

    CB = C // 128      # 8 contraction blocks
    TB = N // 128      # 8 token/key blocks
    VW = 65            # per-head V width (64 dims + ones col)

    with tile.TileContext(nc) as tc:
        with tc.tile_pool(name="misc", bufs=1) as pool_misc, \
             tc.tile_pool(name="w", bufs=1) as pool_w, \
             tc.tile_pool(name="qk", bufs=1) as pool_qk, \
             tc.tile_pool(name="V", bufs=1) as pool_V, \
             tc.tile_pool(name="A", bufs=1) as pool_A, \
             tc.tile_pool(name="PT", bufs=12) as pool_PT, \
             tc.tile_pool(name="norm", bufs=2) as pool_norm, \
             tc.tile_pool(name="outp", bufs=3) as pool_out, \
             tc.tile_pool(name="ps_S", bufs=2, space="PSUM") as ps_S, \
             tc.tile_pool(name="ps_O", bufs=2, space="PSUM") as ps_O, \
             tc.tile_pool(name="ps_fill", bufs=2, space="PSUM") as ps_fill:

            # ---------------- static tiles ----------------
            x_sb = [pool_w.tile([128, N], BF16, tag=f"x{cb}", name=f"x{cb}") for cb in range(CB)]
            wk_sb = [pool_w.tile([128, C], BF16, tag=f"wk{cb}", name=f"wk{cb}") for cb in range(CB)]
            wq_sb = [pool_w.tile([128, C], BF16, tag=f"wq{cb}", name=f"wq{cb}") for cb in range(CB)]
            wv_sb = [pool_w.tile([128, C], BF16, tag=f"wv{cb}", name=f"wv{cb}") for cb in range(CB)]
            wo_sb = [pool_w.tile([128, C], BF16, tag=f"wo{cb}", name=f"wo{cb}") for cb in range(CB)]
            # qk_sb[0..7] = q feature blocks, qk_sb[8..15] = k feature blocks
            qk_sb = [pool_qk.tile([128, N], BF16, tag=f"qk{jb}", name=f"qk{jb}") for jb in range(16)]
            V_sb = [pool_V.tile([128, H * VW], BF16, tag=f"V{tb}", name=f"V{tb}") for tb in range(TB)]
            A_sb = [pool_A.tile([128, N], BF16, tag=f"A{hp}", name=f"A{hp}") for hp in range(CB)]

            bqk_sb = pool_misc.tile([128, 16], F32, tag="bqk")
            bv_sb = pool_misc.tile([1, C], F32, tag="bv")
            bv_rep = pool_misc.tile([128, C], F32, tag="bvrep")
            bo_sb = pool_misc.tile([128, 8], F32, tag="bo")

            # ---------------- DMA loads (multi-queue) ----------------
            # HBM is the startup wall (~350 GB/s/core): order streams by
            # first-use.  sync: x then wq; scalar (free until first exp):
            # wv then wk; gpsimd: biases + wo (not needed until ~150us).
            # The exp-critical 6MB (x, wk, wq) streams first; wv next (first
            # A-weave consumer at ~25us); wo last (needed ~150us); nothing
            # else competes for early HBM bandwidth.
            for cb in range(CB):
                nc.sync.dma_start(x_sb[cb][:, :], xT[cb * 128:(cb + 1) * 128, :])
            for cb in range(CB):
                nc.scalar.dma_start(wk_sb[cb][:, :], wkT[cb * 128:(cb + 1) * 128, :])
            for cb in range(CB):
                nc.sync.dma_start(wq_sb[cb][:, :], wqT[cb * 128:(cb + 1) * 128, :])
            for cb in range(CB):
                nc.scalar.dma_start(wv_sb[cb][:, :], wvT[cb * 128:(cb + 1) * 128, :])
            for tb in range(TB):
                nc.sync.dma_start(V_sb[tb][:, 64::VW], ones_col)
            for cb in range(CB):
                nc.sync.dma_start(wo_sb[cb][:, :], woT[cb * 128:(cb + 1) * 128, :])
            # biases on the gpsimd queue (small)
            nc.gpsimd.dma_start(bqk_sb[:, :], bqk)
            nc.gpsimd.dma_start(bv_sb[:, :], bv)
            nc.gpsimd.dma_start(bo_sb[:, :], bo)
            nc.gpsimd.partition_broadcast(bv_rep[:, :], bv_sb[0:1, :])

            # ---------------- emission helpers ----------------
            def emit_B_chunk(jb, nch):
                """Half of a qk feature block: qk_sb[jb][:, nch*512:]."""
                w_sb = wq_sb if jb < 8 else wk_sb
                jbl = jb % 8
                ps = ps_fill.tile([128, 512], F32, tag="fill")
                for cb in range(CB):
                    nc.tensor.matmul(
                        ps[:, :],
                        w_sb[cb][:, jbl * 128:(jbl + 1) * 128],
                        x_sb[cb][:, nch * 512:(nch + 1) * 512],
                        start=(cb == 0), stop=(cb == CB - 1),
                    )
                nc.vector.tensor_scalar(
                    qk_sb[jb][:, nch * 512:(nch + 1) * 512], ps[:, :],
                    bqk_sb[:, jb:jb + 1], None, mybir.AluOpType.add,
                )

            def emit_B(jb):
                emit_B_chunk(jb, 0)
                emit_B_chunk(jb, 1)

            def emit_A(tb):
                """V natural block tb: V_sb[tb] [128 tokens, 16*65]."""
                for vc in range(2):
                    ps = ps_fill.tile([128, 512], F32, tag="fill")
                    for cb in range(CB):
                        nc.tensor.matmul(
                            ps[:, :],
                            x_sb[cb][:, tb * 128:(tb + 1) * 128],
                            wv_sb[cb][:, vc * 512:(vc + 1) * 512],
                            start=(cb == 0), stop=(cb == CB - 1),
                        )
                    # scatter 8 heads x 64 dims into the 65-strided layout,
                    # adding the broadcast v bias
                    dst = V_sb[tb][:, vc * 8 * VW:(vc + 1) * 8 * VW]
                    dst3 = dst.rearrange("p (h d) -> p h d", h=8)[:, :, 0:64]
                    src3 = ps[:, :].rearrange("p (h d) -> p h d", h=8)
                    bv3 = bv_rep[:, vc * 512:(vc + 1) * 512].rearrange(
                        "p (h d) -> p h d", h=8)
                    nc.vector.tensor_add(dst3, src3, bv3)

            def emit_norm(hp, hh, ic, o_ps):
                """Normalize O (psum [65,512], row 64 = sums) into A_sb."""
                o_cp = pool_norm.tile([VW, 512], F32, tag="ocp", name="ocp")
                nc.vector.tensor_copy(o_cp[:, :], o_ps[:, :])  # frees bank
                # stage sums to partition 0 (partition_broadcast sources the
                # tile's partition 0), broadcast, then reciprocal on the
                # WIDE tile (single-lane DVE reciprocal is ~6 cyc/elem)
                sums0 = pool_norm.tile([1, 512], F32, tag="sums0", name="sums0")
                nc.sync.dma_start(sums0[:, :], o_cp[64:65, :])
                r_rep = pool_norm.tile([64, 512], F32, tag="rrep", name="rrep")
                nc.gpsimd.partition_broadcast(r_rep[:, :], sums0[0:1, :])
                nc.vector.reciprocal(r_rep[:, :], r_rep[:, :])
                if hh == 0:
                    nc.vector.tensor_mul(
                        A_sb[hp][0:64, ic * 512:(ic + 1) * 512],
                        o_cp[0:64, :], r_rep[:, :])
                else:
                    a_tmp = pool_norm.tile([64, 512], BF16, tag="atmp",
                                           name="atmp")
                    nc.vector.tensor_mul(a_tmp[:, :], o_cp[0:64, :], r_rep[:, :])
                    nc.gpsimd.dma_start(
                        A_sb[hp][64:128, ic * 512:(ic + 1) * 512], a_tmp[:, :])

            def emit_att_half(hp, ic, mid=None):
                """One query-chunk (512 cols) of attention pair hp.

                mid: optional callback(kb) emitted between unit kb's exp and
                its PV matmuls -- filler work woven at the point where it
                cannot delay the exp stream but precedes (program-order) any
                consumer that needs it.
                """
                o_ps = {}
                for kb in range(TB):
                    s_ps = ps_S.tile([128, N], F32, tag="S", name="S")
                    for hh in range(2):
                        r0, r1 = hh * 64, hh * 64 + 64
                        nc.tensor.matmul(
                            s_ps[:, hh * 512:(hh + 1) * 512],
                            qk_sb[8 + hp][r0:r1, kb * 128:(kb + 1) * 128],
                            qk_sb[hp][r0:r1, ic * 512:(ic + 1) * 512],
                            start=True, stop=True,
                        )
                    p_t = pool_PT.tile([128, N], BF16, tag="pt", name="pt")
                    nc.scalar.activation(p_t[:, :], s_ps[:, :], EXP,
                                         scale=float(SCALE))
                    if mid is not None:
                        mid(kb)
                    for hh in range(2):
                        h = 2 * hp + hh
                        if kb == 0:
                            o_ps[hh] = ps_O.tile([VW, 512], F32, tag="O",
                                                 name="O")
                        nc.tensor.matmul(
                            o_ps[hh][:, :],
                            V_sb[kb][:, h * VW:(h + 1) * VW],
                            p_t[:, hh * 512:(hh + 1) * 512],
                            start=(kb == 0), stop=(kb == TB - 1),
                        )
                for hh in range(2):
                    emit_norm(hp, hh, ic, o_ps[hh])

            def emit_D(nch):
                """Out-projection for token chunk nch (needs all A_sb[:, nch])."""
                for cb in range(CB):
                    ps = ps_fill.tile([128, 512], F32, tag="fill")
                    for hb in range(CB):
                        nc.tensor.matmul(
                            ps[:, :],
                            wo_sb[hb][:, cb * 128:(cb + 1) * 128],
                            A_sb[hb][:, nch * 512:(nch + 1) * 512],
                            start=(hb == 0), stop=(hb == CB - 1),
                        )
                    o_t = pool_out.tile([128, 512], F32, tag="ot")
                    nc.vector.tensor_scalar(
                        o_t[:, :], ps[:, :], bo_sb[:, cb:cb + 1], None,
                        mybir.AluOpType.add,
                    )
                    nc.sync.dma_start(
                        outT[cb * 128:(cb + 1) * 128,
                             nch * 512:(nch + 1) * 512], o_t[:, :])

            # ---------------- global emission order ----------------
            # Emission order is BOTH program order (dataflow: a reader
            # emitted before a writer sees stale data) and scheduler
            # priority.  Fillers are woven post-exp inside attention units:
            # there they cannot stall the exp stream but still precede
            # (in program order) everything that consumes them.
            emit_B(8)   # pair0 k features
            emit_B(0)   # pair0 q features
            # V blocks 0-1 fill the DMA-bound startup window before the
            # first QK is ready; blocks 2-7 are woven into pair0.ic0.
            emit_A(0)
            emit_A(1)
            emit_att_half(0, 0, mid=lambda kb: emit_A(kb + 2) if kb < 6 else None)
            # pair1 features woven into pair0.ic1
            ic1_fill = {1: lambda: emit_B_chunk(9, 0), 3: lambda: emit_B_chunk(9, 1),
                        5: lambda: emit_B_chunk(1, 0), 7: lambda: emit_B_chunk(1, 1)}
            emit_att_half(0, 1, mid=lambda kb: ic1_fill[kb]() if kb in ic1_fill else None)
            for hp in range(1, 8):
                if hp < 7:
                    # next pair's features woven across this pair's 16 units
                    nj, = [8 + hp + 1],
                    fills = {(0, 2): (8 + hp + 1, 0), (0, 6): (8 + hp + 1, 1),
                             (1, 2): (hp + 1, 0), (1, 6): (hp + 1, 1)}
                    emit_att_half(hp, 0, mid=lambda kb, f=fills, h=hp:
                                  emit_B_chunk(*f[(0, kb)]) if (0, kb) in f else None)
                    emit_att_half(hp, 1, mid=lambda kb, f=fills, h=hp:
                                  emit_B_chunk(*f[(1, kb)]) if (1, kb) in f else None)
                else:
                    emit_att_half(hp, 0)
                    emit_D(0)
                    emit_att_half(hp, 1)
            emit_D(1)

    nc.compile()
    return nc


def _get_nc():
    if "nc" not in _COMPILED:
        _COMPILED["nc"] = _build()
    return _COMPILED["nc"]


def _run(x, in_proj_weight, in_proj_bias, out_proj_weight, out_proj_bias,
         trace=False):
    import ml_dtypes
    from concourse.bass_utils import run_bass_kernel_spmd

    BF = ml_dtypes.bfloat16
    nc = _get_nc()
    x = np.asarray(x, dtype=np.float32)
    w_in = np.asarray(in_proj_weight, dtype=np.float32)
    b_in = np.asarray(in_proj_bias, dtype=np.float32)
    w_out = np.asarray(out_proj_weight, dtype=np.float32)
    b_out = np.asarray(out_proj_bias, dtype=np.float32)

    shared = {
        "wqT": np.ascontiguousarray(w_in[0:C].T).astype(BF),
        "wkT": np.ascontiguousarray(w_in[C:2 * C].T).astype(BF),
        "wvT": np.ascontiguousarray(w_in[2 * C:3 * C].T).astype(BF),
        "woT": np.ascontiguousarray(w_out.T).astype(BF),
        "bqk": np.ascontiguousarray(b_in[0:2 * C].reshape(16, 128).T),
        "bv": np.ascontiguousarray(b_in[2 * C:3 * C])[None, :],
        "bo": np.ascontiguousarray(b_out.reshape(8, 128).T),
        "ones_col": np.ones((128, 16), dtype=BF),
    }
    in_maps = []
    for c in range(NCORES):
        m = dict(shared)
        m["xT"] = np.ascontiguousarray(x[c].T).astype(BF)
        in_maps.append(m)

    res = run_bass_kernel_spmd(nc, in_maps, core_ids=list(range(NCORES)),
                               trace=trace)
    out = np.stack([
        np.ascontiguousarray(res.results[c]["outT"].T) for c in range(NCORES)
    ]).astype(np.float32)
    return out, res


def kernel(x, in_proj_weight, in_proj_bias, out_proj_weight, out_proj_bias):
    out, _ = _run(x, in_proj_weight, in_proj_bias, out_proj_weight,
                  out_proj_bias)
    return out


# revision 16
# speedup vs baseline: 1.0625x; 1.0625x over previous
"""Multi-head attention (B=8, N=1024, C=1024, H=16) on 8 TRN2 NeuronCores.

Strategy: pure data parallelism -- each core computes one batch element with
replicated weights (no collectives). All matmul operands are bf16 (PSUM
accumulation stays fp32), which halves HBM/SBUF traffic and keeps every
tensor resident so the phases can be globally interleaved.

Per-core layout (everything transposed so matmuls contract on partitions):
  xT  [C, N]   = x[b].T          wqT/wkT/wvT [C, C] = in_proj slices .T
  woT [C, C]   = out_proj_weight.T

Emission order = scheduler priority. The exp stream on the Scalar engine
(143us total) is the second-longest resource after the tensor engine
(~190us), so attention pairs are emitted early and projection / out-proj
matmuls are placed after each pair as fillers that soak up the tensor
engine whenever attention is waiting on exp:

  B(k0) B(q0) | [A(V) woven into pair0's units] | pair0 | B(k1) B(q1) |
  pair1 | B(k2) B(q2) | ... | pair7.ic0 | D(nch0) | pair7.ic1 | D(nch1)

Attention per pair hp (heads 2hp, 2hp+1), ic-major (query chunks of 512):
  QK: two row-group-concurrent matmuls (contraction 64) -> S [128k, 1024]
  exp: one ACT op per unit, bf16 out, scale folded in
  PV: V_hat [128k, 65] (64 dims + ones col -> softmax sums in row 64)
  norm: copy out of PSUM, reciprocal of row 64, gpsimd partition
        broadcast, multiply into A_sb (odd heads DMA-shifted to rows 64+)
  D:  out[cb, nch] = sum_hb woT_hb.T @ A_hb + bias
"""
import numpy as np

B, N, C = 8, 1024, 1024
H = 16
HD = C // H               # 64
SCALE = HD ** (-0.5)
NCORES = 8

_COMPILED = {}


def _build():
    import concourse.bass as bass
    import concourse.tile as tile
    from concourse import bacc, mybir

    F32 = mybir.dt.float32
    BF16 = mybir.dt.bfloat16
    EXP = mybir.ActivationFunctionType.Exp

    nc = bacc.Bacc("TRN2", target_bir_lowering=False, debug=False)

    xT = nc.dram_tensor("xT", [C, N], BF16, kind="ExternalInput").ap()
    wqT = nc.dram_tensor("wqT", [C, C], BF16, kind="ExternalInput").ap()
    wkT = nc.dram_tensor("wkT", [C, C], BF16, kind="ExternalInput").ap()
    wvT = nc.dram_tensor("wvT", [C, C], BF16, kind="ExternalInput").ap()
    woT = nc.dram_tensor("woT", [C, C], BF16, kind="ExternalInput").ap()
    bqk = nc.dram_tensor("bqk", [128, 16], F32, kind="ExternalInput").ap()
    bv = nc.dram_tensor("bv", [1, C], F32, kind="ExternalInput").ap()
    bo = nc.dram_tensor("bo", [128, 8], F32, kind="ExternalInput").ap()
    ones_col = nc.dram_tensor("ones_col", [128, 16], BF16, kind="ExternalInput").ap()
    outT = nc.dram_tensor("outT", [C, N], F32, kind="ExternalOutput").ap()

    CB = C // 128      # 8 contraction blocks
    TB = N // 128      # 8 token/key blocks
    VW = 65            # per-head V width (64 dims + ones col)

    with tile.TileContext(nc) as tc:
        with tc.tile_pool(name="misc", bufs=1) as pool_misc, \
             tc.tile_pool(name="w", bufs=1) as pool_w, \
             tc.tile_pool(name="qk", bufs=1) as pool_qk, \
             tc.tile_pool(name="V", bufs=1) as pool_V, \
             tc.tile_pool(name="A", bufs=1) as pool_A, \
             tc.tile_pool(name="PT", bufs=12) as pool_PT, \
             tc.tile_pool(name="norm", bufs=2) as pool_norm, \
             tc.tile_pool(name="outp", bufs=3) as pool_out, \
             tc.tile_pool(name="ps_S", bufs=2, space="PSUM") as ps_S, \
             tc.tile_pool(name="ps_O", bufs=2, space="PSUM") as ps_O, \
             tc.tile_pool(name="ps_fill", bufs=2, space="PSUM") as ps_fill:

            # ---------------- static tiles ----------------
            x_sb = [pool_w.tile([128, N], BF16, tag=f"x{cb}", name=f"x{cb}") for cb in range(CB)]
            wk_sb = [pool_w.tile([128, C], BF16, tag=f"wk{cb}", name=f"wk{cb}") for cb in range(CB)]
            wq_sb = [pool_w.tile([128, C], BF16, tag=f"wq{cb}", name=f"wq{cb}") for cb in range(CB)]
            wv_sb = [pool_w.tile([128, C], BF16, tag=f"wv{cb}", name=f"wv{cb}") for cb in range(CB)]
            wo_sb = [pool_w.tile([128, C], BF16, tag=f"wo{cb}", name=f"wo{cb}") for cb in range(CB)]
            # qk_sb[0..7] = q feature blocks, qk_sb[8..15] = k feature blocks
            qk_sb = [pool_qk.tile([128, N], BF16, tag=f"qk{jb}", name=f"qk{jb}") for jb in range(16)]
            V_sb = [pool_V.tile([128, H * VW], BF16, tag=f"V{tb}", name=f"V{tb}") for tb in range(TB)]
            A_sb = [pool_A.tile([128, N], BF16, tag=f"A{hp}", name=f"A{hp}") for hp in range(CB)]

            bqk_sb = pool_misc.tile([128, 16], F32, tag="bqk")
            bv_sb = pool_misc.tile([1, C], F32, tag="bv")
            bv_rep = pool_misc.tile([128, C], F32, tag="bvrep")
            bo_sb = pool_misc.tile([128, 8], F32, tag="bo")

            # ---------------- DMA loads (multi-queue) ----------------
            # HBM is the startup wall (~350 GB/s/core): order streams by
            # first-use.  sync: x then wq; scalar (free until first exp):
            # wv then wk; gpsimd: biases + wo (not needed until ~150us).
            # Startup is HBM-bandwidth-bound (~350 GB/s/core), so stream
            # strictly by first-use: x||wk (lockstep for B-k), then wq split
            # across both queues, then wv, then wo (needed ~150us).
            for cb in range(CB):
                nc.sync.dma_start(x_sb[cb][:, :], xT[cb * 128:(cb + 1) * 128, :])
            for cb in range(CB):
                nc.scalar.dma_start(wk_sb[cb][:, :], wkT[cb * 128:(cb + 1) * 128, :])
            for cb in range(CB):
                eng = nc.sync if cb % 2 == 0 else nc.scalar
                eng.dma_start(wq_sb[cb][:, :], wqT[cb * 128:(cb + 1) * 128, :])
            for cb in range(CB):
                nc.sync.dma_start(wv_sb[cb][:, :], wvT[cb * 128:(cb + 1) * 128, :])
            for cb in range(CB):
                nc.sync.dma_start(wo_sb[cb][:, :], woT[cb * 128:(cb + 1) * 128, :])
            # biases + V-hat ones columns on the gpsimd queue (small)
            nc.gpsimd.dma_start(bqk_sb[:, :], bqk)
            nc.gpsimd.dma_start(bv_sb[:, :], bv)
            nc.gpsimd.dma_start(bo_sb[:, :], bo)
            for tb in range(TB):
                nc.gpsimd.dma_start(V_sb[tb][:, 64::VW], ones_col)
            nc.gpsimd.partition_broadcast(bv_rep[:, :], bv_sb[0:1, :])

            # ---------------- emission helpers ----------------
            def emit_B_chunk(jb, nch):
                """Half of a qk feature block: qk_sb[jb][:, nch*512:]."""
                w_sb = wq_sb if jb < 8 else wk_sb
                jbl = jb % 8
                ps = ps_fill.tile([128, 512], F32, tag="fill")
                for cb in range(CB):
                    nc.tensor.matmul(
                        ps[:, :],
                        w_sb[cb][:, jbl * 128:(jbl + 1) * 128],
                        x_sb[cb][:, nch * 512:(nch + 1) * 512],
                        start=(cb == 0), stop=(cb == CB - 1),
                    )
                nc.vector.tensor_scalar(
                    qk_sb[jb][:, nch * 512:(nch + 1) * 512], ps[:, :],
                    bqk_sb[:, jb:jb + 1], None, mybir.AluOpType.add,
                )

            def emit_B(jb):
                emit_B_chunk(jb, 0)
                emit_B_chunk(jb, 1)

            def emit_A(tb):
                """V natural block tb: V_sb[tb] [128 tokens, 16*65]."""
                for vc in range(2):
                    ps = ps_fill.tile([128, 512], F32, tag="fill")
                    for cb in range(CB):
                        nc.tensor.matmul(
                            ps[:, :],
                            x_sb[cb][:, tb * 128:(tb + 1) * 128],
                            wv_sb[cb][:, vc * 512:(vc + 1) * 512],
                            start=(cb == 0), stop=(cb == CB - 1),
                        )
                    # scatter 8 heads x 64 dims into the 65-strided layout,
                    # adding the broadcast v bias
                    dst = V_sb[tb][:, vc * 8 * VW:(vc + 1) * 8 * VW]
                    dst3 = dst.rearrange("p (h d) -> p h d", h=8)[:, :, 0:64]
                    src3 = ps[:, :].rearrange("p (h d) -> p h d", h=8)
                    bv3 = bv_rep[:, vc * 512:(vc + 1) * 512].rearrange(
                        "p (h d) -> p h d", h=8)
                    nc.vector.tensor_add(dst3, src3, bv3)

            def emit_norm(hp, hh, ic, o_ps):
                """Normalize O (psum [65,512], row 64 = sums) into A_sb."""
                o_cp = pool_norm.tile([VW, 512], F32, tag="ocp", name="ocp")
                nc.vector.tensor_copy(o_cp[:, :], o_ps[:, :])  # frees bank
                # DVE reciprocal costs ~6.5 cyc per FREE element (partition
                # count is free) -- reshape the 512 sums to [128,4] so the
                # reciprocal is 4 elements/lane, then stage back to
                # partition 0 for partition_broadcast.
                s128 = pool_norm.tile([128, 4], F32, tag="s128", name="s128")
                nc.sync.dma_start(s128[:, :], o_cp[64:65, :])
                nc.vector.reciprocal(s128[:, :], s128[:, :])
                sums0 = pool_norm.tile([1, 512], F32, tag="sums0", name="sums0")
                nc.sync.dma_start(sums0[:, :], s128[:, :])
                r_rep = pool_norm.tile([64, 512], F32, tag="rrep", name="rrep")
                nc.gpsimd.partition_broadcast(r_rep[:, :], sums0[0:1, :])
                if hh == 0:
                    nc.vector.tensor_mul(
                        A_sb[hp][0:64, ic * 512:(ic + 1) * 512],
                        o_cp[0:64, :], r_rep[:, :])
                else:
                    a_tmp = pool_norm.tile([64, 512], BF16, tag="atmp",
                                           name="atmp")
                    nc.vector.tensor_mul(a_tmp[:, :], o_cp[0:64, :], r_rep[:, :])
                    nc.gpsimd.dma_start(
                        A_sb[hp][64:128, ic * 512:(ic + 1) * 512], a_tmp[:, :])

            def emit_att_half(hp, ic, mid=None):
                """One query-chunk (512 cols) of attention pair hp.

                mid: optional callback(kb) emitted between unit kb's exp and
                its PV matmuls -- filler work woven at the point where it
                cannot delay the exp stream but precedes (program-order) any
                consumer that needs it.
                """
                o_ps = {}
                for kb in range(TB):
                    s_ps = ps_S.tile([128, N], F32, tag="S", name="S")
                    for hh in range(2):
                        r0, r1 = hh * 64, hh * 64 + 64
                        nc.tensor.matmul(
                            s_ps[:, hh * 512:(hh + 1) * 512],
                            qk_sb[8 + hp][r0:r1, kb * 128:(kb + 1) * 128],
                            qk_sb[hp][r0:r1, ic * 512:(ic + 1) * 512],
                            start=True, stop=True,
                        )
                    p_t = pool_PT.tile([128, N], BF16, tag="pt", name="pt")
                    nc.scalar.activation(p_t[:, :], s_ps[:, :], EXP,
                                         scale=float(SCALE))
                    if mid is not None:
                        mid(kb)
                    for hh in range(2):
                        h = 2 * hp + hh
                        if kb == 0:
                            o_ps[hh] = ps_O.tile([VW, 512], F32, tag="O",
                                                 name="O")
                        nc.tensor.matmul(
                            o_ps[hh][:, :],
                            V_sb[kb][:, h * VW:(h + 1) * VW],
                            p_t[:, hh * 512:(hh + 1) * 512],
                            start=(kb == 0), stop=(kb == TB - 1),
                        )
                for hh in range(2):
                    emit_norm(hp, hh, ic, o_ps[hh])

            def emit_D(nch):
                """Out-projection for token chunk nch (needs all A_sb[:, nch])."""
                for cb in range(CB):
                    ps = ps_fill.tile([128, 512], F32, tag="fill")
                    for hb in range(CB):
                        nc.tensor.matmul(
                            ps[:, :],
                            wo_sb[hb][:, cb * 128:(cb + 1) * 128],
                            A_sb[hb][:, nch * 512:(nch + 1) * 512],
                            start=(hb == 0), stop=(hb == CB - 1),
                        )
                    o_t = pool_out.tile([128, 512], F32, tag="ot")
                    nc.vector.tensor_scalar(
                        o_t[:, :], ps[:, :], bo_sb[:, cb:cb + 1], None,
                        mybir.AluOpType.add,
                    )
                    nc.sync.dma_start(
                        outT[cb * 128:(cb + 1) * 128,
                             nch * 512:(nch + 1) * 512], o_t[:, :])

            # ---------------- global emission order ----------------
            # Emission order is BOTH program order (dataflow: a reader
            # emitted before a writer sees stale data) and scheduler
            # priority.  Fillers are woven post-exp inside attention units:
            # there they cannot stall the exp stream but still precede
            # (in program order) everything that consumes them.
            emit_B(8)   # pair0 k features
            emit_B(0)   # pair0 q features
            # V blocks 0-1 fill the DMA-bound startup window before the
            # first QK is ready; blocks 2-7 are woven into pair0.ic0.
            emit_A(0)
            emit_A(1)
            emit_att_half(0, 0, mid=lambda kb: emit_A(kb + 2) if kb < 6 else None)
            # pair1 features woven into pair0.ic1
            ic1_fill = {1: lambda: emit_B_chunk(9, 0), 3: lambda: emit_B_chunk(9, 1),
                        5: lambda: emit_B_chunk(1, 0), 7: lambda: emit_B_chunk(1, 1)}
            emit_att_half(0, 1, mid=lambda kb: ic1_fill[kb]() if kb in ic1_fill else None)
            for hp in range(1, 8):
                if hp < 7:
                    # next pair's features woven across this pair's 16 units
                    nj, = [8 + hp + 1],
                    fills = {(0, 2): (8 + hp + 1, 0), (0, 6): (8 + hp + 1, 1),
                             (1, 2): (hp + 1, 0), (1, 6): (hp + 1, 1)}
                    emit_att_half(hp, 0, mid=lambda kb, f=fills, h=hp:
                                  emit_B_chunk(*f[(0, kb)]) if (0, kb) in f else None)
                    emit_att_half(hp, 1, mid=lambda kb, f=fills, h=hp:
                                  emit_B_chunk(*f[(1, kb)]) if (1, kb) in f else None)
                else:
                    emit_att_half(hp, 0)
                    emit_D(0)
                    emit_att_half(hp, 1)
            emit_D(1)

    nc.compile()
    return nc


def _get_nc():
    if "nc" not in _COMPILED:
        _COMPILED["nc"] = _build()
    return _COMPILED["nc"]


def _run(x, in_proj_weight, in_proj_bias, out_proj_weight, out_proj_bias,
         trace=False):
    import ml_dtypes
    from concourse.bass_utils import run_bass_kernel_spmd

    BF = ml_dtypes.bfloat16
    nc = _get_nc()
    x = np.asarray(x, dtype=np.float32)
    w_in = np.asarray(in_proj_weight, dtype=np.float32)
    b_in = np.asarray(in_proj_bias, dtype=np.float32)
    w_out = np.asarray(out_proj_weight, dtype=np.float32)
    b_out = np.asarray(out_proj_bias, dtype=np.float32)

    shared = {
        "wqT": np.ascontiguousarray(w_in[0:C].T).astype(BF),
        "wkT": np.ascontiguousarray(w_in[C:2 * C].T).astype(BF),
        "wvT": np.ascontiguousarray(w_in[2 * C:3 * C].T).astype(BF),
        "woT": np.ascontiguousarray(w_out.T).astype(BF),
        "bqk": np.ascontiguousarray(b_in[0:2 * C].reshape(16, 128).T),
        "bv": np.ascontiguousarray(b_in[2 * C:3 * C])[None, :],
        "bo": np.ascontiguousarray(b_out.reshape(8, 128).T),
        "ones_col": np.ones((128, 16), dtype=BF),
    }
    in_maps = []
    for c in range(NCORES):
        m = dict(shared)
        m["xT"] = np.ascontiguousarray(x[c].T).astype(BF)
        in_maps.append(m)

    res = run_bass_kernel_spmd(nc, in_maps, core_ids=list(range(NCORES)),
                               trace=trace)
    out = np.stack([
        np.ascontiguousarray(res.results[c]["outT"].T) for c in range(NCORES)
    ]).astype(np.float32)
    return out, res


def kernel(x, in_proj_weight, in_proj_bias, out_proj_weight, out_proj_bias):
    out, _ = _run(x, in_proj_weight, in_proj_bias, out_proj_weight,
                  out_proj_bias)
    return out


# revision 25
# speedup vs baseline: 1.0755x; 1.0122x over previous
"""Multi-head attention (B=8, N=1024, C=1024, H=16) on 8 TRN2 NeuronCores.

Strategy: pure data parallelism -- each core computes one batch element with
replicated weights (no collectives). All matmul operands are bf16 (PSUM
accumulation stays fp32), which halves HBM/SBUF traffic and keeps every
tensor resident so the phases can be globally interleaved.

Per-core layout (everything transposed so matmuls contract on partitions):
  xT  [C, N]   = x[b].T          wqT/wkT/wvT [C, C] = in_proj slices .T
  woT [C, C]   = out_proj_weight.T

Emission order = scheduler priority. The exp stream on the Scalar engine
(143us total) is the second-longest resource after the tensor engine
(~190us), so attention pairs are emitted early and projection / out-proj
matmuls are placed after each pair as fillers that soak up the tensor
engine whenever attention is waiting on exp:

  B(k0) B(q0) | [A(V) woven into pair0's units] | pair0 | B(k1) B(q1) |
  pair1 | B(k2) B(q2) | ... | pair7.ic0 | D(nch0) | pair7.ic1 | D(nch1)

Attention per pair hp (heads 2hp, 2hp+1), ic-major (query chunks of 512):
  QK: two row-group-concurrent matmuls (contraction 64) -> S [128k, 1024]
  exp: one ACT op per unit, bf16 out, scale folded in
  PV: V_hat [128k, 65] (64 dims + ones col -> softmax sums in row 64)
  norm: copy out of PSUM, reciprocal of row 64, gpsimd partition
        broadcast, multiply into A_sb (odd heads DMA-shifted to rows 64+)
  D:  out[cb, nch] = sum_hb woT_hb.T @ A_hb + bias
"""
import numpy as np

B, N, C = 8, 1024, 1024
H = 16
HD = C // H               # 64
SCALE = HD ** (-0.5)
NCORES = 8

_COMPILED = {}


def _build():
    import concourse.bass as bass
    import concourse.tile as tile
    from concourse import bacc, mybir

    F32 = mybir.dt.float32
    BF16 = mybir.dt.bfloat16
    EXP = mybir.ActivationFunctionType.Exp

    nc = bacc.Bacc("TRN2", target_bir_lowering=False, debug=False)

    xT = nc.dram_tensor("xT", [C, N], BF16, kind="ExternalInput").ap()
    wqT = nc.dram_tensor("wqT", [C, C], BF16, kind="ExternalInput").ap()
    wkT = nc.dram_tensor("wkT", [C, C], BF16, kind="ExternalInput").ap()
    wvT = nc.dram_tensor("wvT", [C, C], BF16, kind="ExternalInput").ap()
    woT = nc.dram_tensor("woT", [C, C], BF16, kind="ExternalInput").ap()
    bqk = nc.dram_tensor("bqk", [128, 16], F32, kind="ExternalInput").ap()
    bv = nc.dram_tensor("bv", [1, C], F32, kind="ExternalInput").ap()
    bo = nc.dram_tensor("bo", [128, 8], F32, kind="ExternalInput").ap()
    ones_col = nc.dram_tensor("ones_col", [128, 16], BF16, kind="ExternalInput").ap()
    outT = nc.dram_tensor("outT", [C, N], F32, kind="ExternalOutput").ap()

    CB = C // 128      # 8 contraction blocks
    TB = N // 128      # 8 token/key blocks
    VW = 65            # per-head V width (64 dims + ones col)

    with tile.TileContext(nc) as tc:
        with tc.tile_pool(name="misc", bufs=1) as pool_misc, \
             tc.tile_pool(name="w", bufs=1) as pool_w, \
             tc.tile_pool(name="qk", bufs=1) as pool_qk, \
             tc.tile_pool(name="V", bufs=1) as pool_V, \
             tc.tile_pool(name="A", bufs=1) as pool_A, \
             tc.tile_pool(name="PT", bufs=12) as pool_PT, \
             tc.tile_pool(name="norm", bufs=2) as pool_norm, \
             tc.tile_pool(name="outp", bufs=3) as pool_out, \
             tc.tile_pool(name="ps_S", bufs=2, space="PSUM") as ps_S, \
             tc.tile_pool(name="ps_O", bufs=2, space="PSUM") as ps_O, \
             tc.tile_pool(name="ps_fill", bufs=2, space="PSUM") as ps_fill:

            # ---------------- static tiles ----------------
            x_sb = [pool_w.tile([128, N], BF16, tag=f"x{cb}", name=f"x{cb}") for cb in range(CB)]
            wk_sb = [pool_w.tile([128, C], BF16, tag=f"wk{cb}", name=f"wk{cb}") for cb in range(CB)]
            wq_sb = [pool_w.tile([128, C], BF16, tag=f"wq{cb}", name=f"wq{cb}") for cb in range(CB)]
            wv_sb = [pool_w.tile([128, C], BF16, tag=f"wv{cb}", name=f"wv{cb}") for cb in range(CB)]
            wo_sb = [pool_w.tile([128, C], BF16, tag=f"wo{cb}", name=f"wo{cb}") for cb in range(CB)]
            # qk_sb[0..7] = q feature blocks, qk_sb[8..15] = k feature blocks
            qk_sb = [pool_qk.tile([128, N], BF16, tag=f"qk{jb}", name=f"qk{jb}") for jb in range(16)]
            V_sb = [pool_V.tile([128, H * VW], BF16, tag=f"V{tb}", name=f"V{tb}") for tb in range(TB)]
            A_sb = [pool_A.tile([128, N], BF16, tag=f"A{hp}", name=f"A{hp}") for hp in range(CB)]

            bqk_sb = pool_misc.tile([128, 16], F32, tag="bqk")
            bv_sb = pool_misc.tile([1, C], F32, tag="bv")
            bv_rep = pool_misc.tile([128, C], F32, tag="bvrep")
            bo_sb = pool_misc.tile([128, 8], F32, tag="bo")

            # ---------------- DMA loads (multi-queue) ----------------
            # HBM is the startup wall (~350 GB/s/core): order streams by
            # first-use.  sync: x then wq; scalar (free until first exp):
            # wv then wk; gpsimd: biases + wo (not needed until ~150us).
            # Startup is HBM-bandwidth-bound (~350 GB/s/core), so stream
            # strictly by first-use: x||wk (lockstep for B-k), then wv||wq
            # (wv early so the 27us of V-production can start; it gates the
            # whole pair0 PV trail), then wo (needed ~150us).
            for cb in range(CB):
                nc.sync.dma_start(x_sb[cb][:, :], xT[cb * 128:(cb + 1) * 128, :])
            for cb in range(CB):
                nc.scalar.dma_start(wk_sb[cb][:, :], wkT[cb * 128:(cb + 1) * 128, :])
            for cb in range(CB):
                nc.sync.dma_start(wv_sb[cb][:, :], wvT[cb * 128:(cb + 1) * 128, :])
            for cb in range(CB):
                nc.scalar.dma_start(wq_sb[cb][:, :], wqT[cb * 128:(cb + 1) * 128, :])
            for cb in range(CB):
                nc.sync.dma_start(wo_sb[cb][:, :], woT[cb * 128:(cb + 1) * 128, :])
            # biases + V-hat ones columns on the gpsimd queue (small)
            nc.gpsimd.dma_start(bqk_sb[:, :], bqk)
            nc.gpsimd.dma_start(bv_sb[:, :], bv)
            nc.gpsimd.dma_start(bo_sb[:, :], bo)
            for tb in range(TB):
                nc.gpsimd.dma_start(V_sb[tb][:, 64::VW], ones_col)
            nc.gpsimd.partition_broadcast(bv_rep[:, :], bv_sb[0:1, :])

            # ---------------- emission helpers ----------------
            def B_fill_mm(jb, nch, cb, state):
                """One matmul of a qk feature chunk (weavable filler)."""
                w_sb = wq_sb if jb < 8 else wk_sb
                jbl = jb % 8
                if cb == 0:
                    state["ps"] = ps_fill.tile([128, 512], F32, tag="fill", name="fill")
                nc.tensor.matmul(
                    state["ps"][:, :],
                    w_sb[cb][:, jbl * 128:(jbl + 1) * 128],
                    x_sb[cb][:, nch * 512:(nch + 1) * 512],
                    start=(cb == 0), stop=(cb == CB - 1),
                )
                if cb == CB - 1:
                    nc.vector.tensor_scalar(
                        qk_sb[jb][:, nch * 512:(nch + 1) * 512], state["ps"][:, :],
                        bqk_sb[:, jb:jb + 1], None, mybir.AluOpType.add,
                    )

            def B_fillers(jb):
                """16 single-matmul closures computing feature block jb."""
                state0, state1 = {}, {}
                return [
                    (lambda nch=nch, cb=cb, st=(state0 if nch == 0 else state1):
                     B_fill_mm(jb, nch, cb, st))
                    for nch in range(2) for cb in range(CB)
                ]

            def emit_B(jb):
                for f in B_fillers(jb):
                    f()

            def emit_A(tb):
                """V natural block tb: V_sb[tb] [128 tokens, 16*65]."""
                for vc in range(2):
                    ps = ps_fill.tile([128, 512], F32, tag="fill", name="fill")
                    for cb in range(CB):
                        nc.tensor.matmul(
                            ps[:, :],
                            x_sb[cb][:, tb * 128:(tb + 1) * 128],
                            wv_sb[cb][:, vc * 512:(vc + 1) * 512],
                            start=(cb == 0), stop=(cb == CB - 1),
                        )
                    # scatter 8 heads x 64 dims into the 65-strided layout,
                    # adding the broadcast v bias
                    dst = V_sb[tb][:, vc * 8 * VW:(vc + 1) * 8 * VW]
                    dst3 = dst.rearrange("p (h d) -> p h d", h=8)[:, :, 0:64]
                    src3 = ps[:, :].rearrange("p (h d) -> p h d", h=8)
                    bv3 = bv_rep[:, vc * 512:(vc + 1) * 512].rearrange(
                        "p (h d) -> p h d", h=8)
                    nc.vector.tensor_add(dst3, src3, bv3)

            def emit_norm(hp, hh, ic, o_ps):
                """Normalize O (psum [65,512], row 64 = sums) into A_sb."""
                o_cp = pool_norm.tile([VW, 512], F32, tag="ocp", name="ocp")
                nc.vector.tensor_copy(o_cp[:, :], o_ps[:, :])  # frees bank
                # DVE reciprocal costs ~6.5 cyc per FREE element (partition
                # count is free) -- reshape the 512 sums to [128,4] so the
                # reciprocal is 4 elements/lane, then stage back to
                # partition 0 for partition_broadcast.
                s128 = pool_norm.tile([128, 4], F32, tag="s128", name="s128")
                nc.sync.dma_start(s128[:, :], o_cp[64:65, :])
                nc.vector.reciprocal(s128[:, :], s128[:, :])
                sums0 = pool_norm.tile([1, 512], F32, tag="sums0", name="sums0")
                nc.sync.dma_start(sums0[:, :], s128[:, :])
                r_rep = pool_norm.tile([64, 512], F32, tag="rrep", name="rrep")
                nc.gpsimd.partition_broadcast(r_rep[:, :], sums0[0:1, :])
                if hh == 0:
                    nc.vector.tensor_mul(
                        A_sb[hp][0:64, ic * 512:(ic + 1) * 512],
                        o_cp[0:64, :], r_rep[:, :])
                else:
                    a_tmp = pool_norm.tile([64, 512], BF16, tag="atmp",
                                           name="atmp")
                    nc.vector.tensor_mul(a_tmp[:, :], o_cp[0:64, :], r_rep[:, :])
                    nc.gpsimd.dma_start(
                        A_sb[hp][64:128, ic * 512:(ic + 1) * 512], a_tmp[:, :])

            def emit_att_half(hp, ic, mid=None, fillers=None, per_unit=2):
                """One query-chunk (512 cols) of attention pair hp.

                mid: optional callback(kb) emitted between unit kb's exp and
                its PV matmuls.  fillers: list of single-matmul closures
                woven `per_unit` at a time at the same point -- fine enough
                granularity that the next unit's QK is never stalled by more
                than ~one matmul.  Both precede (program-order) anything
                that consumes their output.
                """
                fillers = list(fillers) if fillers else []
                o_ps = {}
                for kb in range(TB):
                    s_ps = ps_S.tile([128, N], F32, tag="S", name="S")
                    for hh in range(2):
                        r0, r1 = hh * 64, hh * 64 + 64
                        nc.tensor.matmul(
                            s_ps[:, hh * 512:(hh + 1) * 512],
                            qk_sb[8 + hp][r0:r1, kb * 128:(kb + 1) * 128],
                            qk_sb[hp][r0:r1, ic * 512:(ic + 1) * 512],
                            start=True, stop=True,
                        )
                    p_t = pool_PT.tile([128, N], BF16, tag="pt", name="pt")
                    nc.scalar.activation(p_t[:, :], s_ps[:, :], EXP,
                                         scale=float(SCALE))
                    if mid is not None:
                        mid(kb)
                    for _ in range(per_unit):
                        if fillers:
                            fillers.pop(0)()
                    for hh in range(2):
                        h = 2 * hp + hh
                        if kb == 0:
                            o_ps[hh] = ps_O.tile([VW, 512], F32, tag="O",
                                                 name="O")
                        nc.tensor.matmul(
                            o_ps[hh][:, :],
                            V_sb[kb][:, h * VW:(h + 1) * VW],
                            p_t[:, hh * 512:(hh + 1) * 512],
                            start=(kb == 0), stop=(kb == TB - 1),
                        )
                for hh in range(2):
                    emit_norm(hp, hh, ic, o_ps[hh])
                for f in fillers:  # flush leftovers (low priority tail)
                    f()

            def D_fill_mm(nch, cb, hb, state):
                """One matmul of an out-projection chunk (weavable filler)."""
                if hb == 0:
                    state["ps"] = ps_fill.tile([128, 512], F32, tag="fill", name="fill")
                nc.tensor.matmul(
                    state["ps"][:, :],
                    wo_sb[hb][:, cb * 128:(cb + 1) * 128],
                    A_sb[hb][:, nch * 512:(nch + 1) * 512],
                    start=(hb == 0), stop=(hb == CB - 1),
                )
                if hb == CB - 1:
                    o_t = pool_out.tile([128, 512], F32, tag="ot", name="ot")
                    nc.vector.tensor_scalar(
                        o_t[:, :], state["ps"][:, :], bo_sb[:, cb:cb + 1], None,
                        mybir.AluOpType.add,
                    )
                    nc.sync.dma_start(
                        outT[cb * 128:(cb + 1) * 128,
                             nch * 512:(nch + 1) * 512], o_t[:, :])

            def D_fillers(nch):
                """64 single-matmul closures for out-proj token chunk nch."""
                states = [dict() for _ in range(CB)]
                return [
                    (lambda nch=nch, cb=cb, hb=hb:
                     D_fill_mm(nch, cb, hb, states[cb]))
                    for cb in range(CB) for hb in range(CB)
                ]

            def emit_D(nch):
                for f in D_fillers(nch):
                    f()

            # ---------------- global emission order ----------------
            # Emission order is BOTH program order (dataflow: a reader
            # emitted before a writer sees stale data) and scheduler
            # priority.  Fillers are woven post-exp inside attention units:
            # there they cannot stall the exp stream but still precede
            # (in program order) everything that consumes them.
            emit_B(8)   # pair0 k features
            emit_B(0)   # pair0 q features
            # V blocks 0-1 fill the DMA-bound startup window before the
            # first QK is ready; blocks 2-7 are woven into pair0.ic0
            # (pair0 is tensor-bound on V production regardless).
            emit_A(0)
            emit_A(1)
            emit_att_half(0, 0, mid=lambda kb: emit_A(kb + 2) if kb < 6 else None)
            # pair1 features woven into pair0.ic1 at 4 matmuls/unit
            emit_att_half(0, 1, fillers=B_fillers(9) + B_fillers(1), per_unit=4)
            for hp in range(1, 7):
                # next pair's features woven 2 matmuls/unit across 16 units
                nxt = B_fillers(8 + hp + 1) + B_fillers(hp + 1)
                emit_att_half(hp, 0, fillers=nxt[:16], per_unit=2)
                emit_att_half(hp, 1, fillers=nxt[16:], per_unit=2)
            emit_att_half(7, 0)
            # out-proj nch0 woven into pair7.ic1 (4/unit + flush); nch1 tails
            emit_att_half(7, 1, fillers=D_fillers(0), per_unit=4)
            emit_D(1)

    nc.compile()
    return nc


def _get_nc():
    if "nc" not in _COMPILED:
        _COMPILED["nc"] = _build()
    return _COMPILED["nc"]


def _run(x, in_proj_weight, in_proj_bias, out_proj_weight, out_proj_bias,
         trace=False):
    import ml_dtypes
    from concourse.bass_utils import run_bass_kernel_spmd

    BF = ml_dtypes.bfloat16
    nc = _get_nc()
    x = np.asarray(x, dtype=np.float32)
    w_in = np.asarray(in_proj_weight, dtype=np.float32)
    b_in = np.asarray(in_proj_bias, dtype=np.float32)
    w_out = np.asarray(out_proj_weight, dtype=np.float32)
    b_out = np.asarray(out_proj_bias, dtype=np.float32)

    shared = {
        "wqT": np.ascontiguousarray(w_in[0:C].T).astype(BF),
        "wkT": np.ascontiguousarray(w_in[C:2 * C].T).astype(BF),
        "wvT": np.ascontiguousarray(w_in[2 * C:3 * C].T).astype(BF),
        "woT": np.ascontiguousarray(w_out.T).astype(BF),
        "bqk": np.ascontiguousarray(b_in[0:2 * C].reshape(16, 128).T),
        "bv": np.ascontiguousarray(b_in[2 * C:3 * C])[None, :],
        "bo": np.ascontiguousarray(b_out.reshape(8, 128).T),
        "ones_col": np.ones((128, 16), dtype=BF),
    }
    in_maps = []
    for c in range(NCORES):
        m = dict(shared)
        m["xT"] = np.ascontiguousarray(x[c].T).astype(BF)
        in_maps.append(m)

    res = run_bass_kernel_spmd(nc, in_maps, core_ids=list(range(NCORES)),
                               trace=trace)
    out = np.stack([
        np.ascontiguousarray(res.results[c]["outT"].T) for c in range(NCORES)
    ]).astype(np.float32)
    return out, res


def kernel(x, in_proj_weight, in_proj_bias, out_proj_weight, out_proj_bias):
    out, _ = _run(x, in_proj_weight, in_proj_bias, out_proj_weight,
                  out_proj_bias)
    return out
